# revision 9
# baseline (speedup 1.0000x reference)
"""Trainium2 Bass kernel for CausalWanSelfAttention (frame-causal windowed
attention with QK-RMSNorm + RoPE), sharded over 8 NeuronCores.

Sharding: each core owns T = (h*w)/8 tokens of every frame (frame-balanced
interleave).  Each core computes Q/K/V projections + RMSNorm + RoPE for its
own tokens, K/V are exchanged with two AllGathers, attention + O-projection
are computed locally for the core's query tokens.

Device layouts:
  - q/k feature-major [ch, tok] (channels on partitions), with each head's
    128 channels permuted to [re(0..63) | im(0..63)] so RoPE works on
    contiguous partition blocks (permutation is folded into Wq/Wk on host).
  - v token-major [tok, ch] (natural channel order).
  - scores computed as s^T [keys, q]  ->  softmax denominator via
    ones-matmul (partition reduction on the tensor engine), exp on ACT.
  - attention out o^T [ch, tok]; per-head 1/Z applied by DVE during PSUM
    eviction with a partition-broadcast tile.
  - RMSNorm scale r (per token) is folded into the RoPE cos/sin tables
    (scalar multiplication commutes with rotation); per-channel gain g and
    bias b are folded into the ACT eviction (per-partition scale/bias).
"""

import math
import sys
from contextlib import ExitStack

import numpy as np

if "/opt/trn_rl_repo" not in sys.path:
    sys.path.insert(0, "/opt/trn_rl_repo")

import ml_dtypes

BF16 = ml_dtypes.bfloat16
NC = 8  # cores
D = 128  # head dim
EPS = 1e-6


# ---------------------------------------------------------------------------
# helpers
# ---------------------------------------------------------------------------
def _pieces(lo, hi, T):
    """Split the global (within-frame) token range [lo, hi) into per-core
    pieces.  Returns [(core, a, b)] with a/b local to the core's frame-chunk."""
    out = []
    c = lo // T
    while lo < hi:
        b = min(hi, (c + 1) * T)
        out.append((c, lo - c * T, b - c * T))
        lo = b
        c += 1
    return out


def _segs(q0, S, bank=512):
    """Split [q0, S) at multiples of `bank` -> list of absolute (qa, qb)."""
    pts = [q0]
    nxt = (q0 // bank + 1) * bank
    while nxt < S:
        pts.append(nxt)
        nxt += bank
    pts.append(S)
    return [(pts[i], pts[i + 1]) for i in range(len(pts) - 1)]


def _chunks(frame_len, width=128):
    return [(g * width, min(frame_len, (g + 1) * width))
            for g in range((frame_len + width - 1) // width)]


# ---------------------------------------------------------------------------
# device program
# ---------------------------------------------------------------------------
_BUILD_CACHE = {}


def build_program(NH, F, T, allowed_kf, cap_waits=True, debug=False):
    """Build the SPMD Bass program (identical on all 8 cores).

    NH: number of heads; F: frames; T: tokens per (core, frame);
    allowed_kf[qf] = list of key frames query-frame qf may attend to
    (must make, for each kf, the attending q-set a contiguous suffix of
    frames -- true for causal masks).
    """
    key = (NH, F, T, tuple(tuple(a) for a in allowed_kf), cap_waits, debug)
    if key in _BUILD_CACHE:
        return _BUILD_CACHE[key]

    import concourse.bass as bass
    import concourse.mybir as mybir
    import concourse.tile as tile
    from concourse.mybir import ActivationFunctionType as AF

    dt = mybir.dt
    DIM = NH * D
    S = F * T              # tokens per core
    FRAME = NC * T         # tokens per frame
    NHALF = 2
    H0 = (S + 1) // 2      # token halves for the q/k projections
    SLICE = min(512, DIM)  # out-channel slice for v/o projections
    NSL = DIM // SLICE
    TOKCH = _chunks(S, 128)  # token chunks for v/o projections

    # for each key frame kf: the first query frame that attends to it, and
    # check the q-set is a suffix
    first_qf = {}
    for kf in range(F):
        qs = [qf for qf in range(F) if kf in allowed_kf[qf]]
        assert qs, f"key frame {kf} unused"
        assert qs == list(range(qs[0], F)), "non-suffix q-set unsupported"
        first_qf[kf] = qs[0]

    nc = bass.Bass()

    # ---------------- I/O ----------------
    xT_d = nc.dram_tensor("xT", [DIM, S], dt.bfloat16, kind="ExternalInput")
    w_d = {}
    for nm in ("wqT", "wkT", "wvT", "woT"):
        w_d[nm] = nc.dram_tensor(nm, [DIM, DIM], dt.bfloat16, kind="ExternalInput")
    # packed per-channel affine constants: bq|gq|bq*gq|bk|gk|bk*gk
    bias_d = nc.dram_tensor("bias_pack", [128, 6 * NH], dt.float32,
                            kind="ExternalInput")
    bv_d = nc.dram_tensor("bv_r", [1, DIM], dt.bfloat16, kind="ExternalInput")
    bo_d = nc.dram_tensor("bo_r", [1, DIM], dt.float32, kind="ExternalInput")
    angS_d = nc.dram_tensor("angS", [128, S], dt.float32, kind="ExternalInput")
    angC_d = nc.dram_tensor("angC", [128, S], dt.float32, kind="ExternalInput")
    out_d = nc.dram_tensor("out", [S, DIM], dt.float32, kind="ExternalOutput")

    rg = [list(range(NC))]
    inv_sqrt_d = 1.0 / math.sqrt(D)

    GF = NC * T            # global keys per frame (1560)
    KVSZ = 2 * DIM * S     # flat kv block per core (k feature-major + v token-major)

    with tile.TileContext(nc) as tc, ExitStack() as ctx:
        dram = ctx.enter_context(tc.tile_pool(name="dram", bufs=1, space="DRAM"))
        kv_loc = dram.tile([1, KVSZ], dt.bfloat16)
        kv_all = dram.tile([NC, KVSZ], dt.bfloat16, addr_space="Shared")

        const = ctx.enter_context(tc.tile_pool(name="const", bufs=1))
        resid = ctx.enter_context(tc.tile_pool(name="resid", bufs=1))

        ones_key = const.tile([128, 1], dt.bfloat16)
        nc.vector.memset(ones_key, 1.0)
        ones_row = const.tile([1, 128], dt.bfloat16)
        nc.vector.memset(ones_row, 1.0)
        negpi = const.tile([128, 1], dt.float32)
        nc.vector.memset(negpi, -math.pi)
        eps_t = const.tile([128, 1], dt.float32)
        nc.vector.memset(eps_t, EPS)

        # constant / bias tiles (one DMA for the packed affine constants)
        bias_sb = const.tile([128, 6 * NH], dt.float32)
        nc.sync.dma_start(out=bias_sb[:], in_=bias_d[:])
        bq_sb = bias_sb[:, 0 * NH:1 * NH]
        gq_sb = bias_sb[:, 1 * NH:2 * NH]
        bqgq_sb = bias_sb[:, 2 * NH:3 * NH]
        bk_sb = bias_sb[:, 3 * NH:4 * NH]
        gk_sb = bias_sb[:, 4 * NH:5 * NH]
        bkgk_sb = bias_sb[:, 5 * NH:6 * NH]
        bv_sb = const.tile([1, DIM], dt.bfloat16)
        nc.sync.dma_start(out=bv_sb[:], in_=bv_d[:])
        bo_bc = const.tile([128, DIM], dt.float32)
        nc.sync.dma_start(
            out=bo_bc[:],
            in_=bass.AP(tensor=bo_d[:].tensor, offset=bo_d[:].offset,
                        ap=[[0, 128]] + bo_d[:].ap[1:]),
        )

        # x (feature-major), resident
        xT_sb = resid.tile([128, NH, S], dt.bfloat16)
        nc.sync.dma_start(out=xT_sb[:], in_=xT_d[:].rearrange("(m p) s -> p m s", p=128))

        # raw RoPE sin/cos (shared q/k)
        angS_sb = resid.tile([128, S], dt.float32)
        angC_sb = resid.tile([128, S], dt.float32)
        nc.sync.dma_start(out=angS_sb[:], in_=angS_d[:])
        nc.sync.dma_start(out=angC_sb[:], in_=angC_d[:])
        # angles arrive host-canonicalized to [-pi, pi] (ACT Sin table range)
        sin_raw = resid.tile([128, S], dt.float32)
        cos_raw = resid.tile([128, S], dt.float32)
        nc.scalar.activation(sin_raw[:], angS_sb[:], AF.Sin)
        nc.scalar.activation(cos_raw[:], angC_sb[:], AF.Sin)

        qhat = resid.tile([128, NH, S], dt.bfloat16)
        khat = resid.tile([128, NH, S], dt.bfloat16)
        qrot = resid.tile([128, NH, S], dt.bfloat16)
        krot = resid.tile([128, NH, S], dt.bfloat16)
        r_q = resid.tile([1, S], dt.float32)
        r_k = resid.tile([1, S], dt.float32)
        oT_sb = resid.tile([128, NH, S], dt.bfloat16)

        halves = [(0, H0), (H0, S)] if S > H0 else [(0, S)]

        # ---------------- Q/K projections + RMS stats ----------------
        def qk_proj(wname, bias_sb, gain_sb, bg_sb, hat, r_sb):
          with ExitStack() as pctx:
            wpool = pctx.enter_context(tc.tile_pool(name=f"w_{wname}", bufs=1))
            pspool = pctx.enter_context(
                tc.tile_pool(name=f"ps_{wname}", bufs=4, space="PSUM"))
            sspool = pctx.enter_context(
                tc.tile_pool(name=f"ss_{wname}", bufs=2, space="PSUM"))
            evpool = pctx.enter_context(tc.tile_pool(name=f"ev_{wname}", bufs=3))
            w_sb = wpool.tile([128, NH, DIM], dt.bfloat16, name=f"wsb_{wname}")
            nc.sync.dma_start(
                out=w_sb[:], in_=w_d[wname][:].rearrange("(kc p) n -> p kc n", p=128))
            ss_ps = {}
            for hi, (ha, hb) in enumerate(halves):
                ss_ps[hi] = sspool.tile([1, hb - ha], dt.float32, tag="ss", name=f"ss{hi}")
            for m in range(NH):
                ps = {}
                for hi, (ha, hb) in enumerate(halves):
                    ps[hi] = pspool.tile([128, hb - ha], dt.float32, tag="ps", name=f"ps{hi}")
                for kc in range(NH):
                    for hi, (ha, hb) in enumerate(halves):
                        nc.tensor.matmul(ps[hi][:, :hb - ha],
                                         w_sb[:, kc, m * 128:(m + 1) * 128],
                                         xT_sb[:, kc, ha:hb],
                                         start=(kc == 0), stop=(kc == NH - 1))
                for hi, (ha, hb) in enumerate(halves):
                    hw_ = hb - ha
                    sq = evpool.tile([128, H0], dt.bfloat16, tag="sq")
                    # (q + b)^2
                    nc.scalar.activation(sq[:, :hw_], ps[hi][:, :hw_], AF.Square,
                                         bias=bias_sb[:, m:m + 1])
                    # qhat = (q + b) * g = q*g + b*g
                    nc.scalar.activation(hat[:, m, ha:hb], ps[hi][:, :hw_],
                                         AF.Identity, bias=bg_sb[:, m:m + 1],
                                         scale=gain_sb[:, m:m + 1])
                    nc.tensor.matmul(ss_ps[hi][0:1, :hw_], ones_key[:],
                                     sq[:, :hw_],
                                     start=(m == 0), stop=(m == NH - 1))
            for hi, (ha, hb) in enumerate(halves):
                hw_ = hb - ha
                rt = evpool.tile([1, H0], dt.float32, tag="rt")
                # sqrt(mean(q^2) + eps)
                nc.scalar.activation(rt[0:1, :hw_], ss_ps[hi][0:1, :hw_], AF.Sqrt,
                                     bias=eps_t[0:1, :], scale=1.0 / DIM)
                nc.vector.reciprocal(r_sb[0:1, ha:hb], rt[0:1, :hw_])

        # ---------------- RoPE ----------------
        def rope(hat, rot, r_sb, tag):
          with ExitStack() as pctx:
            rp = pctx.enter_context(tc.tile_pool(name=f"rope_{tag}", bufs=3))
            r_dram = dram.tile([1, S], dt.float32, name=f"rdram_{tag}")
            nc.sync.dma_start(out=r_dram[:], in_=r_sb[0:1, :])
            rb = resid.tile([128, S], dt.float32, name=f"rb_{tag}")
            nc.sync.dma_start(
                out=rb[:],
                in_=bass.AP(tensor=r_dram.tensor, offset=r_dram[0:1, :].offset,
                            ap=[[0, 128]] + r_dram[0:1, :].ap[1:]))
            ct = resid.tile([128, S], dt.bfloat16, name=f"cos_{tag}")
            st = resid.tile([128, S], dt.bfloat16, name=f"sin_{tag}")
            nc.vector.tensor_mul(ct[:], cos_raw[:], rb[:])
            nc.vector.tensor_mul(st[:], sin_raw[:], rb[:])
            for m in range(NH):
                sw = rp.tile([128, S], dt.bfloat16, tag="sw")
                nc.sync.dma_start(out=sw[0:64, :], in_=hat[64:128, m, :])
                nc.sync.dma_start(out=sw[64:128, :], in_=hat[0:64, m, :])
                t1 = rp.tile([128, S], dt.bfloat16, tag="t1")
                t2 = rp.tile([128, S], dt.bfloat16, tag="t2")
                nc.vector.tensor_mul(t1[:], hat[:, m, :], ct[:])
                nc.vector.tensor_mul(t2[:], sw[:], st[:])
                nc.vector.tensor_add(rot[:, m, :], t1[:], t2[:])

        # ---------------- V projection (token-major) ----------------
        def v_proj():
          with ExitStack() as pctx:
            wpool = pctx.enter_context(tc.tile_pool(name="w_v", bufs=1))
            pspool = pctx.enter_context(
                tc.tile_pool(name="ps_v", bufs=len(TOKCH) + 1, space="PSUM"))
            evpool = pctx.enter_context(tc.tile_pool(name="ev_v", bufs=3))
            w_sb = wpool.tile([128, NH, DIM], dt.bfloat16, name="wsb_v")
            nc.sync.dma_start(
                out=w_sb[:], in_=w_d["wvT"][:].rearrange("(kc p) n -> p kc n", p=128))
            for sl in range(NSL):
                ps = {}
                for ti in range(len(TOKCH)):
                    ps[ti] = pspool.tile([128, SLICE], dt.float32, tag="vps", name=f"vps{ti}")
                for kc in range(NH):
                    for ti, (ta, tb) in enumerate(TOKCH):
                        nc.tensor.matmul(ps[ti][:tb - ta, :], xT_sb[:, kc, ta:tb],
                                         w_sb[:, kc, sl * SLICE:(sl + 1) * SLICE],
                                         start=(kc == 0), stop=False)
                for ti, (ta, tb) in enumerate(TOKCH):
                    tw = tb - ta
                    nc.tensor.matmul(ps[ti][:tw, :], ones_row[0:1, :tw],
                                     bv_sb[0:1, sl * SLICE:(sl + 1) * SLICE],
                                     start=False, stop=True)
                    vt = evpool.tile([128, SLICE], dt.bfloat16, tag="vev")
                    nc.scalar.activation(vt[:tw, :], ps[ti][:tw, :], AF.Copy)
                    nc.sync.dma_start(
                        out=bass.AP(
                            tensor=kv_loc.tensor,
                            offset=kv_loc[:].offset + DIM * S + ta * DIM
                            + sl * SLICE,
                            ap=[[DIM, tw], [1, SLICE]]),
                        in_=vt[:tw, :])

        # ---- phase order: K first, V next, then ONE fused AllGather (the
        # collective-bandwidth curve strongly favors a single big transfer),
        # then Q (overlaps the gather) ----
        qk_proj("wkT", bk_sb, gk_sb, bkgk_sb, khat, r_k)
        rope(khat, krot, r_k, "k")
        for m in range(NH):
            nc.sync.dma_start(
                out=bass.AP(tensor=kv_loc.tensor,
                            offset=kv_loc[:].offset + m * 128 * S,
                            ap=[[S, 128], [1, S]]),
                in_=krot[:, m, :])
        v_proj()
        nc.gpsimd.collective_compute(
            "AllGather", mybir.AluOpType.bypass, ins=[kv_loc[:]],
            outs=[kv_all[:]], replica_groups=rg)
        qk_proj("wqT", bq_sb, gq_sb, bqgq_sb, qhat, r_q)
        rope(qhat, qrot, r_q, "q")

        # ---------------- attention ----------------
        # Keys are processed in GLOBAL frame order (contiguous across source
        # cores) so every chunk is a full 128 keys (13 chunks / frame instead
        # of 16).  Per head pair: score tiles [128, 2(hi), W<=390] live in a
        # 2-bank PSUM tile (hi at col 0 / 512) so exp is ONE ACT instruction
        # covering both heads.  q blocks are frame-aligned; o and z accumulate
        # per query-frame (o: [128, 2, T] bank per qf; z: [1, 2, T] rows of a
        # shared bank), evicted as soon as kf == qf completes.
        actx = ctx.enter_context(ExitStack())
        att_k = actx.enter_context(tc.tile_pool(name="att_k", bufs=2))
        att_v = actx.enter_context(tc.tile_pool(name="att_v", bufs=2))
        att_s = actx.enter_context(tc.tile_pool(name="att_s", bufs=2, space="PSUM"))
        att_o = actx.enter_context(tc.tile_pool(name="att_o", bufs=1, space="PSUM"))
        att_z = actx.enter_context(tc.tile_pool(name="att_z", bufs=1, space="PSUM"))
        att_p = actx.enter_context(tc.tile_pool(name="att_p", bufs=4))
        att_m = actx.enter_context(tc.tile_pool(name="att_m", bufs=2))

        KCH_G = _chunks(GF, 128)          # 13 chunks of global frame keys
        NJ = len(KCH_G)
        # v-load pieces: (core, t0, t1, chunk j, partition p0) covering one frame
        vpieces = []
        for c in range(NC):
            g = c * T
            while g < (c + 1) * T:
                j = g // 128
                ge = min((c + 1) * T, (j + 1) * 128)
                vpieces.append((c, g - c * T, ge - c * T, j, g - j * 128))
                g = ge
        # q blocks per key frame: [q0, S) in pieces of <=2*T, frame-aligned
        QBLK = {}
        for kf in range(F):
            q0 = T * first_qf[kf]
            blocks = []
            while q0 < S:
                q1 = min(q0 + 2 * T, S)
                blocks.append((q0, q1))
                q0 = q1
            QBLK[kf] = blocks

        NPG = NH // 2
        for pg in range(NPG):
            o_ps = {}
            for qf in range(F):
                o_ps[qf] = att_o.tile([128, 512], dt.float32, tag=f"o{qf}",
                                      name=f"o{qf}")
            z_ps = att_z.tile([128, 512], dt.float32, tag="z", name="z")

            for kf in range(F):
                kr_t = att_k.tile([128, 2, GF], dt.bfloat16, tag="kr")
                for hi in range(2):
                    nc.sync.dma_start(
                        out=kr_t[:, hi, :],
                        in_=bass.AP(
                            tensor=kv_all.tensor,
                            offset=kv_all[:].offset
                            + ((pg * 2 + hi) * 128) * S + kf * T,
                            ap=[[S, 128], [KVSZ, NC], [1, T]]))
                v_t = att_v.tile([128, NJ, 2, 128], dt.bfloat16, tag="vt")
                for (c, t0, t1, j, p0) in vpieces:
                    nc.sync.dma_start(
                        out=v_t[p0:p0 + (t1 - t0), j, :, :],
                        in_=bass.AP(
                            tensor=kv_all.tensor,
                            offset=kv_all[:].offset + c * KVSZ + DIM * S
                            + (kf * T + t0) * DIM + pg * 256,
                            ap=[[DIM, t1 - t0], [1, 256]]))
                for j, (ga, gb) in enumerate(KCH_G):
                    kw = gb - ga
                    for (qa, qb) in QBLK[kf]:
                        W = qb - qa
                        s_t = att_s.tile([128, 1024], dt.float32, tag="s")
                        for hi in range(2):
                            nc.tensor.matmul(
                                s_t[:kw, hi * 512:hi * 512 + W],
                                kr_t[:, hi, ga:gb],
                                qrot[:, 2 * pg + hi, qa:qb],
                                start=True, stop=True)
                        p_t = att_p.tile([128, 2, 2 * T], dt.bfloat16, tag="p")
                        sv = s_t[:].rearrange("p (two x) -> p two x", two=2)
                        nc.scalar.activation(p_t[:kw, :, :W], sv[:kw, :, :W],
                                             AF.Exp, scale=inv_sqrt_d)
                        for qf in range(qa // T, (qb - 1) // T + 1):
                            oa = qf * T - qa
                            first = (kf == 0 and j == 0)
                            last = (kf == qf and j == NJ - 1)
                            nc.tensor.matmul(
                                z_ps[32 * qf:32 * qf + 1, 0:2 * T],
                                ones_key[:kw, :],
                                p_t[:kw, :, oa:oa + T],
                                start=first, stop=last)
                            for hi in range(2):
                                # start=True zeroes the whole bank row, so only
                                # hi0 opens the group; hi1 accumulates onto the
                                # zeroed right half (program order guarantees
                                # hi0's start executes first)
                                nc.tensor.matmul(
                                    o_ps[qf][:, hi * T:(hi + 1) * T],
                                    v_t[:kw, j, hi, :],
                                    p_t[:kw, hi, oa:oa + T],
                                    start=(first and hi == 0), stop=last,
                                    skip_group_check=True)
                # query frame kf is complete: normalize + evict
                qf = kf
                z_sb = att_m.tile([1, 2 * T], dt.float32, tag="zsb", name="zsb")
                z_dram = dram.tile([2, S], dt.float32, tag="zdram", bufs=2,
                                   name="zdram")
                nc.scalar.activation(z_sb[0:1, :],
                                     z_ps[32 * qf:32 * qf + 1, 0:2 * T],
                                     AF.Copy)
                nc.vector.reciprocal(z_sb[0:1, :], z_sb[0:1, :])
                nc.sync.dma_start(
                    out=bass.AP(tensor=z_dram.tensor,
                                offset=z_dram[:].offset + qf * T,
                                ap=[[S, 2], [1, T]]),
                    in_=z_sb[0:1, :])
                izb = att_m.tile([128, 2, T], dt.float32, tag="izb", name="izb")
                nc.sync.dma_start(
                    out=izb[:],
                    in_=bass.AP(tensor=z_dram.tensor,
                                offset=z_dram[:].offset + qf * T,
                                ap=[[0, 128], [S, 2], [1, T]]))
                for hi in range(2):
                    nc.vector.tensor_mul(
                        oT_sb[:, 2 * pg + hi, qf * T:(qf + 1) * T],
                        o_ps[qf][:, hi * T:(hi + 1) * T],
                        izb[:, hi, :])

        actx.close()  # release attention PSUM banks before the O-projection

        if debug:
            dbg_oT = nc.dram_tensor("dbg_oT", [128, NH * S], dt.bfloat16,
                                    kind="ExternalOutput")
            dbg_kr = nc.dram_tensor("dbg_kr", [128, NH * S], dt.bfloat16,
                                    kind="ExternalOutput")
            dbg_qr = nc.dram_tensor("dbg_qr", [128, NH * S], dt.bfloat16,
                                    kind="ExternalOutput")
            nc.sync.dma_start(out=dbg_oT[:], in_=oT_sb[:].rearrange("p m s -> p (m s)"))
            nc.sync.dma_start(out=dbg_kr[:], in_=krot[:].rearrange("p m s -> p (m s)"))
            nc.sync.dma_start(out=dbg_qr[:], in_=qrot[:].rearrange("p m s -> p (m s)"))

        # ---------------- O projection ----------------
        wpool = ctx.enter_context(tc.tile_pool(name="w_o", bufs=3))
        pspool = ctx.enter_context(
            tc.tile_pool(name="ps_o", bufs=len(TOKCH) + 1, space="PSUM"))
        evpool = ctx.enter_context(tc.tile_pool(name="ev_o", bufs=3))
        for sl in range(NSL):
            ps = {}
            for ti in range(len(TOKCH)):
                ps[ti] = pspool.tile([128, SLICE], dt.float32, tag="ops", name=f"ops{ti}")
            for m in range(NH):
                wt = wpool.tile([128, SLICE], dt.bfloat16, tag="wo")
                nc.sync.dma_start(
                    out=wt[:],
                    in_=w_d["woT"][m * 128:(m + 1) * 128,
                                   sl * SLICE:(sl + 1) * SLICE])
                for ti, (ta, tb) in enumerate(TOKCH):
                    nc.tensor.matmul(ps[ti][:tb - ta, :], oT_sb[:, m, ta:tb],
                                     wt[:], start=(m == 0), stop=(m == NH - 1))
            for ti, (ta, tb) in enumerate(TOKCH):
                tw = tb - ta
                ot = evpool.tile([128, SLICE], dt.float32, tag="oev")
                nc.vector.tensor_add(ot[:tw, :], ps[ti][:tw, :],
                                     bo_bc[:tw, sl * SLICE:(sl + 1) * SLICE])
                nc.sync.dma_start(
                    out=out_d[ta:tb, sl * SLICE:(sl + 1) * SLICE],
                    in_=ot[:tw, :])

    if cap_waits:
        _cap_sync_waits(nc, mybir)
    _BUILD_CACHE[key] = nc
    return nc


def _cap_sync_waits(nc, mybir, cap=1):
    """Walrus engine-instruction structs only have a limited number of sync
    wait slots.  Hoist excess waits onto InstNoOp carriers placed immediately
    before the instruction on the same engine stream."""
    exempt = (mybir.InstNoOp, mybir.InstEventSemaphore,
              mybir.InstAllEngineBarrier)
    for f in nc.m.functions:
        for bb in f.blocks:
            out = []
            changed = False
            for inst in bb.instructions:
                si = inst.sync_info
                if (si is None or len(si.on_wait) <= cap
                        or isinstance(inst, exempt)):
                    out.append(inst)
                    continue
                waits = list(si.on_wait)
                keep, excess = waits[:cap], waits[cap:]
                while excess:
                    batch, excess = excess[:cap], excess[cap:]
                    out.append(mybir.InstNoOp(
                        name=f"{inst.name}-w{len(out)}",
                        engine=inst.engine,
                        bass_nofuse=True,
                        sync_info=mybir.SyncInfo(on_wait=batch, on_update=[]),
                    ))
                inst.sync_info = mybir.SyncInfo(on_wait=keep,
                                                on_update=list(si.on_update))
                out.append(inst)
                changed = True
            if changed:
                bb.instructions = out


# ---------------------------------------------------------------------------
# host side
# ---------------------------------------------------------------------------
def _perm(NH):
    p = np.empty(NH * D, np.int64)
    for hh in range(NH):
        base = hh * D
        for j in range(D // 2):
            p[base + j] = base + 2 * j
            p[base + D // 2 + j] = base + 2 * j + 1
    return p


def _host_inputs(x, freqs, Wq, bq, Wk, bk, Wv, bv, Wo, bo, gq, gk,
                 f, h, w, num_heads, local_attn_size, sink_size, start_frame):
    NH = num_heads
    DIM = NH * D
    FRAME = h * w
    assert FRAME % NC == 0
    T = FRAME // NC
    S = f * T
    perm = _perm(NH)

    def bf(a):
        return np.ascontiguousarray(a, dtype=np.float32).astype(BF16)

    wqT = bf(Wq[perm].T)
    wkT = bf(Wk[perm].T)
    wvT = bf(Wv.T)
    woT = bf(Wo.T)
    def chunkmajor(a):
        return np.asarray(a, np.float32)[perm].reshape(NH, D).T
    bias_pack = np.ascontiguousarray(np.concatenate(
        [chunkmajor(bq), chunkmajor(gq), chunkmajor(bq) * chunkmajor(gq),
         chunkmajor(bk), chunkmajor(gk), chunkmajor(bk) * chunkmajor(gk)],
        axis=1), np.float32)
    bv_r = bf(bv.reshape(1, DIM))
    bo_r = np.ascontiguousarray(bo.reshape(1, DIM), np.float32)

    c = D // 2
    c1 = c // 3
    c0 = c - 2 * c1
    freqs = np.asarray(freqs, np.float32)

    in_maps = []
    tok_idx = []
    for core in range(NC):
        idx = np.concatenate(
            [fr * FRAME + T * core + np.arange(T) for fr in range(f)])
        tok_idx.append(idx)
        xT = bf(np.asarray(x[0], np.float32)[idx].T)
        fr = idx // FRAME
        rem = idx % FRAME
        hh_i = rem // w
        ww_i = rem % w
        ang = np.empty((c, S), np.float32)
        ang[:c0, :] = freqs[start_frame + fr][:, :c0].T
        ang[c0:c0 + c1, :] = freqs[hh_i][:, c0:c0 + c1].T
        ang[c0 + c1:, :] = freqs[ww_i][:, c0 + c1:c].T
        def wrap(a):
            a = np.asarray(a, np.float64)
            return (a - 2 * np.pi * np.round(a / (2 * np.pi))).astype(np.float32)
        # top half encodes -sin via the (ang + pi) phase shift
        angS = np.ascontiguousarray(
            np.concatenate([wrap(ang + np.pi), wrap(ang)], 0), np.float32)
        angC = np.ascontiguousarray(
            np.concatenate([wrap(ang + np.pi / 2), wrap(ang + np.pi / 2)], 0),
            np.float32)
        in_maps.append({
            "xT": xT, "wqT": wqT, "wkT": wkT, "wvT": wvT, "woT": woT,
            "bias_pack": bias_pack,
            "bv_r": bv_r, "bo_r": bo_r, "angS": angS, "angC": angC,
        })
    return in_maps, tok_idx, T, S


def _allowed(f, local_attn_size, sink_size):
    return [
        [kf for kf in range(f)
         if kf <= qf and (qf - kf < local_attn_size or kf < sink_size)]
        for qf in range(f)
    ]


def kernel(x, freqs, Wq, bq, Wk, bk, Wv, bv, Wo, bo, gq, gk,
           f, h, w, num_heads, local_attn_size, sink_size, start_frame,
           _trace=False):
    from concourse.bass_utils import run_bass_kernel_spmd

    f = int(f); h = int(h); w = int(w)
    num_heads = int(num_heads)
    local_attn_size = int(local_attn_size)
    sink_size = int(sink_size)
    start_frame = int(start_frame)

    x = np.asarray(x)
    B, L, DIM = x.shape
    assert B == 1 and DIM == num_heads * D

    allowed = _allowed(f, local_attn_size, sink_size)
    in_maps, tok_idx, T, S = _host_inputs(
        x, freqs, Wq, bq, Wk, bk, Wv, bv, Wo, bo, gq, gk,
        f, h, w, num_heads, local_attn_size, sink_size, start_frame)
    nc = build_program(num_heads, f, T, allowed)
    res = run_bass_kernel_spmd(nc, in_maps, core_ids=list(range(NC)),
                               trace=_trace)
    out = np.empty((1, L, DIM), np.float32)
    for core in range(NC):
        out[0, tok_idx[core]] = res.results[core]["out"]
    if _trace:
        kernel._last_results = res
    return out



# revision 23
# speedup vs baseline: 3.4556x; 3.4556x over previous
"""Trainium2 Bass kernel for CausalWanSelfAttention (frame-causal windowed
attention with QK-RMSNorm + RoPE), sharded over 8 NeuronCores.

Sharding: each core owns T = (h*w)/8 tokens of every frame (frame-balanced
interleave).  Each core computes Q/K/V projections + RMSNorm + RoPE for its
own tokens, K/V are exchanged with two AllGathers, attention + O-projection
are computed locally for the core's query tokens.

Device layouts:
  - q/k feature-major [ch, tok] (channels on partitions), with each head's
    128 channels permuted to [re(0..63) | im(0..63)] so RoPE works on
    contiguous partition blocks (permutation is folded into Wq/Wk on host).
  - v token-major [tok, ch] (natural channel order).
  - scores computed as s^T [keys, q]  ->  softmax denominator via
    ones-matmul (partition reduction on the tensor engine), exp on ACT.
  - attention out o^T [ch, tok]; per-head 1/Z applied by DVE during PSUM
    eviction with a partition-broadcast tile.
  - RMSNorm scale r (per token) is folded into the RoPE cos/sin tables
    (scalar multiplication commutes with rotation); per-channel gain g and
    bias b are folded into the ACT eviction (per-partition scale/bias).
"""

import math
import sys
from contextlib import ExitStack

import numpy as np

if "/opt/trn_rl_repo" not in sys.path:
    sys.path.insert(0, "/opt/trn_rl_repo")

import ml_dtypes

BF16 = ml_dtypes.bfloat16
NC = 8  # cores
D = 128  # head dim
EPS = 1e-6


# ---------------------------------------------------------------------------
# helpers
# ---------------------------------------------------------------------------
def _pieces(lo, hi, T):
    """Split the global (within-frame) token range [lo, hi) into per-core
    pieces.  Returns [(core, a, b)] with a/b local to the core's frame-chunk."""
    out = []
    c = lo // T
    while lo < hi:
        b = min(hi, (c + 1) * T)
        out.append((c, lo - c * T, b - c * T))
        lo = b
        c += 1
    return out


def _segs(q0, S, bank=512):
    """Split [q0, S) at multiples of `bank` -> list of absolute (qa, qb)."""
    pts = [q0]
    nxt = (q0 // bank + 1) * bank
    while nxt < S:
        pts.append(nxt)
        nxt += bank
    pts.append(S)
    return [(pts[i], pts[i + 1]) for i in range(len(pts) - 1)]


def _chunks(frame_len, width=128):
    return [(g * width, min(frame_len, (g + 1) * width))
            for g in range((frame_len + width - 1) // width)]


# ---------------------------------------------------------------------------
# device program
# ---------------------------------------------------------------------------
_BUILD_CACHE = {}


def build_program(NH, F, T, allowed_kf, cap_waits=True, debug=False):
    """Build the SPMD Bass program (identical on all 8 cores).

    NH: number of heads; F: frames; T: tokens per (core, frame);
    allowed_kf[qf] = list of key frames query-frame qf may attend to
    (must make, for each kf, the attending q-set a contiguous suffix of
    frames -- true for causal masks).
    """
    key = (NH, F, T, tuple(tuple(a) for a in allowed_kf), cap_waits, debug)
    if key in _BUILD_CACHE:
        return _BUILD_CACHE[key]

    import concourse.bass as bass
    import concourse.mybir as mybir
    import concourse.tile as tile
    from concourse.mybir import ActivationFunctionType as AF

    dt = mybir.dt
    DIM = NH * D
    S = F * T              # tokens per core
    FRAME = NC * T         # tokens per frame
    NHALF = 2
    H0 = (S + 1) // 2      # token halves for the q/k projections
    SLICE = min(512, DIM)  # out-channel slice for v/o projections
    NSL = DIM // SLICE
    TOKCH = _chunks(S, 128)  # token chunks for v/o projections

    # for each key frame kf: the first query frame that attends to it, and
    # check the q-set is a suffix
    first_qf = {}
    for kf in range(F):
        qs = [qf for qf in range(F) if kf in allowed_kf[qf]]
        assert qs, f"key frame {kf} unused"
        assert qs == list(range(qs[0], F)), "non-suffix q-set unsupported"
        first_qf[kf] = qs[0]

    nc = bass.Bass()

    # ---------------- I/O ----------------
    xT_d = nc.dram_tensor("xT", [DIM, S], dt.bfloat16, kind="ExternalInput")
    w_d = {}
    for nm in ("wqT", "wkT", "wvT", "woT"):
        w_d[nm] = nc.dram_tensor(nm, [DIM, DIM], dt.bfloat16, kind="ExternalInput")
    # packed per-channel affine constants: bq|gq|bq*gq|bk|gk|bk*gk
    bias_d = nc.dram_tensor("bias_pack", [128, 6 * NH], dt.float32,
                            kind="ExternalInput")
    perm_d = nc.dram_tensor("perm128", [128, 128], dt.bfloat16,
                            kind="ExternalInput")
    bv_d = nc.dram_tensor("bv_r", [1, DIM], dt.bfloat16, kind="ExternalInput")
    bo_d = nc.dram_tensor("bo_r", [1, DIM], dt.float32, kind="ExternalInput")
    angS_d = nc.dram_tensor("angS", [128, S], dt.float32, kind="ExternalInput")
    angC_d = nc.dram_tensor("angC", [128, S], dt.float32, kind="ExternalInput")
    out_d = nc.dram_tensor("out", [S, DIM], dt.float32, kind="ExternalOutput")

    rg = [list(range(NC))]
    inv_sqrt_d = 1.0 / math.sqrt(D)

    GF = NC * T            # global keys per frame (1560)
    KVSZ = 2 * DIM * S     # flat kv block per core (k feature-major + v token-major)

    with tile.TileContext(nc) as tc, ExitStack() as ctx:
        dram = ctx.enter_context(tc.tile_pool(name="dram", bufs=1, space="DRAM"))
        kv_loc = dram.tile([1, KVSZ], dt.bfloat16)
        kv_all = dram.tile([NC, KVSZ], dt.bfloat16, addr_space="Shared")

        const = ctx.enter_context(tc.tile_pool(name="const", bufs=1))
        resid = ctx.enter_context(tc.tile_pool(name="resid", bufs=1))

        ones_key = const.tile([128, 1], dt.bfloat16)
        nc.vector.memset(ones_key, 1.0)
        ones_row = const.tile([1, 128], dt.bfloat16)
        nc.vector.memset(ones_row, 1.0)
        perm_sb = const.tile([128, 128], dt.bfloat16)
        nc.sync.dma_start(out=perm_sb[:], in_=perm_d[:])
        negpi = const.tile([128, 1], dt.float32)
        nc.vector.memset(negpi, -math.pi)
        eps_t = const.tile([128, 1], dt.float32)
        nc.vector.memset(eps_t, EPS)

        # constant / bias tiles (one DMA for the packed affine constants)
        bias_sb = const.tile([128, 6 * NH], dt.float32)
        nc.sync.dma_start(out=bias_sb[:], in_=bias_d[:])
        bq_sb = bias_sb[:, 0 * NH:1 * NH]
        gq_sb = bias_sb[:, 1 * NH:2 * NH]
        bqgq_sb = bias_sb[:, 2 * NH:3 * NH]
        bk_sb = bias_sb[:, 3 * NH:4 * NH]
        gk_sb = bias_sb[:, 4 * NH:5 * NH]
        bkgk_sb = bias_sb[:, 5 * NH:6 * NH]
        bv_sb = const.tile([1, DIM], dt.bfloat16)
        nc.sync.dma_start(out=bv_sb[:], in_=bv_d[:])
        bo_bc = const.tile([128, DIM], dt.float32)
        nc.sync.dma_start(
            out=bo_bc[:],
            in_=bass.AP(tensor=bo_d[:].tensor, offset=bo_d[:].offset,
                        ap=[[0, 128]] + bo_d[:].ap[1:]),
        )

        # x (feature-major), resident
        xT_sb = resid.tile([128, NH, S], dt.bfloat16)
        nc.sync.dma_start(out=xT_sb[:], in_=xT_d[:].rearrange("(m p) s -> p m s", p=128))

        # raw RoPE sin/cos (shared q/k)
        angS_sb = resid.tile([128, S], dt.float32)
        angC_sb = resid.tile([128, S], dt.float32)
        nc.sync.dma_start(out=angS_sb[:], in_=angS_d[:])
        nc.sync.dma_start(out=angC_sb[:], in_=angC_d[:])
        # angles arrive host-canonicalized to [-pi, pi] (ACT Sin table range)
        sin_raw = resid.tile([128, S], dt.float32)
        cos_raw = resid.tile([128, S], dt.float32)
        nc.scalar.activation(sin_raw[:], angS_sb[:], AF.Sin)
        nc.scalar.activation(cos_raw[:], angC_sb[:], AF.Sin)

        qhat = resid.tile([128, NH, S], dt.bfloat16)
        khat = resid.tile([128, NH, S], dt.bfloat16)
        qrot = resid.tile([128, NH, S], dt.bfloat16)
        krot = resid.tile([128, NH, S], dt.bfloat16)
        r_q = resid.tile([1, S], dt.bfloat16)
        r_k = resid.tile([1, S], dt.bfloat16)
        oT_sb = resid.tile([128, NH, S], dt.bfloat16)

        halves = [(0, H0), (H0, S)] if S > H0 else [(0, S)]

        # ---------------- Q/K projections + RMS stats ----------------
        def qk_proj(wname, bias_sb, gain_sb, bg_sb, hat, r_sb):
          with ExitStack() as pctx:
            wpool = pctx.enter_context(tc.tile_pool(name=f"w_{wname}", bufs=3))
            pspool = pctx.enter_context(
                tc.tile_pool(name=f"ps_{wname}", bufs=4, space="PSUM"))
            sspool = pctx.enter_context(
                tc.tile_pool(name=f"ss_{wname}", bufs=2, space="PSUM"))
            evpool = pctx.enter_context(tc.tile_pool(name=f"ev_{wname}", bufs=3))
            wsrc = w_d[wname][:].rearrange("(kc p) n -> p kc n", p=128)
            ss_ps = {}
            for hi, (ha, hb) in enumerate(halves):
                ss_ps[hi] = sspool.tile([1, hb - ha], dt.float32, tag="ss", name=f"ss{hi}")
            for m in range(NH):
                # per-head weight slice: small DMAs prefetch via pool rotation
                w_m = wpool.tile([128, NH, 128], dt.bfloat16, tag="w")
                nc.sync.dma_start(out=w_m[:],
                                  in_=wsrc[:, :, m * 128:(m + 1) * 128])
                ps = {}
                for hi, (ha, hb) in enumerate(halves):
                    ps[hi] = pspool.tile([128, hb - ha], dt.float32, tag="ps", name=f"ps{hi}")
                for kc in range(NH):
                    for hi, (ha, hb) in enumerate(halves):
                        nc.tensor.matmul(ps[hi][:, :hb - ha],
                                         w_m[:, kc, :],
                                         xT_sb[:, kc, ha:hb],
                                         start=(kc == 0), stop=(kc == NH - 1))
                for hi, (ha, hb) in enumerate(halves):
                    hw_ = hb - ha
                    sq = evpool.tile([128, H0], dt.bfloat16, tag="sq")
                    # (q + b)^2
                    nc.scalar.activation(sq[:, :hw_], ps[hi][:, :hw_], AF.Square,
                                         bias=bias_sb[:, m:m + 1])
                    # qhat = (q + b) * g = q*g + b*g
                    nc.scalar.activation(hat[:, m, ha:hb], ps[hi][:, :hw_],
                                         AF.Identity, bias=bg_sb[:, m:m + 1],
                                         scale=gain_sb[:, m:m + 1])
                    nc.tensor.matmul(ss_ps[hi][0:1, :hw_], ones_key[:],
                                     sq[:, :hw_],
                                     start=(m == 0), stop=(m == NH - 1))
            for hi, (ha, hb) in enumerate(halves):
                hw_ = hb - ha
                rt = evpool.tile([1, H0], dt.float32, tag="rt")
                # sqrt(mean(q^2) + eps)
                nc.scalar.activation(rt[0:1, :hw_], ss_ps[hi][0:1, :hw_], AF.Sqrt,
                                     bias=eps_t[0:1, :], scale=1.0 / DIM)
                with nc.allow_low_precision(reason="1/rms folded into bf16 rope tables"):
                    nc.vector.reciprocal(r_sb[0:1, ha:hb], rt[0:1, :hw_])

        # ---------------- RoPE ----------------
        # The half-swap and the 1/rms broadcast both run on the (otherwise
        # idle) tensor engine: sw = P @ hat with a host-provided permutation,
        # rb = ones^T @ r.  Matmul outputs split at the 512-col psum bank edge.
        PSEG = [(a, min(a + 512, S)) for a in range(0, S, 512)]

        def rope(hat, rot, r_sb, tag):
          with ExitStack() as pctx:
            rp = pctx.enter_context(tc.tile_pool(name=f"rope_{tag}", bufs=3))
            rps = pctx.enter_context(
                tc.tile_pool(name=f"rps_{tag}", bufs=1, space="PSUM"))
            rb_ps = rps.tile([128, 1024], dt.float32, tag="rb", name=f"rb_{tag}")
            for (a, b) in PSEG:
                nc.tensor.matmul(rb_ps[:, a:b], ones_row[:], r_sb[0:1, a:b],
                                 start=True, stop=True)
            ct = resid.tile([128, S], dt.bfloat16, name=f"cos_{tag}")
            st = resid.tile([128, S], dt.bfloat16, name=f"sin_{tag}")
            nc.vector.tensor_mul(ct[:], cos_raw[:], rb_ps[:, :S])
            nc.vector.tensor_mul(st[:], sin_raw[:], rb_ps[:, :S])
            for m in range(NH):
                sw_ps = rps.tile([128, 1024], dt.float32, tag="sw",
                                 name=f"sw_{tag}{m}", bufs=2)
                for (a, b) in PSEG:
                    nc.tensor.matmul(sw_ps[:, a:b], perm_sb[:],
                                     hat[:, m, a:b], start=True, stop=True)
                t1 = rp.tile([128, S], dt.bfloat16, tag="t1")
                t2 = rp.tile([128, S], dt.bfloat16, tag="t2")
                nc.vector.tensor_mul(t1[:], hat[:, m, :], ct[:])
                nc.vector.tensor_mul(t2[:], sw_ps[:, :S], st[:])
                nc.vector.tensor_add(rot[:, m, :], t1[:], t2[:])

        # ---------------- V projection (token-major) ----------------
        def v_proj():
          with ExitStack() as pctx:
            wpool = pctx.enter_context(tc.tile_pool(name="w_v", bufs=2))
            pspool = pctx.enter_context(
                tc.tile_pool(name="ps_v", bufs=len(TOKCH) + 1, space="PSUM"))
            evpool = pctx.enter_context(tc.tile_pool(name="ev_v", bufs=3))
            wvsrc = w_d["wvT"][:].rearrange("(kc p) n -> p kc n", p=128)
            for sl in range(NSL):
                w_sb = wpool.tile([128, NH, SLICE], dt.bfloat16, tag="wv")
                nc.sync.dma_start(
                    out=w_sb[:],
                    in_=wvsrc[:, :, sl * SLICE:(sl + 1) * SLICE])
                ps = {}
                for ti in range(len(TOKCH)):
                    ps[ti] = pspool.tile([128, SLICE], dt.float32, tag="vps", name=f"vps{ti}")
                for kc in range(NH):
                    for ti, (ta, tb) in enumerate(TOKCH):
                        nc.tensor.matmul(ps[ti][:tb - ta, :], xT_sb[:, kc, ta:tb],
                                         w_sb[:, kc, :],
                                         start=(kc == 0), stop=False)
                for ti, (ta, tb) in enumerate(TOKCH):
                    tw = tb - ta
                    nc.tensor.matmul(ps[ti][:tw, :], ones_row[0:1, :tw],
                                     bv_sb[0:1, sl * SLICE:(sl + 1) * SLICE],
                                     start=False, stop=True)
                    vt = evpool.tile([128, SLICE], dt.bfloat16, tag="vev")
                    nc.scalar.activation(vt[:tw, :], ps[ti][:tw, :], AF.Copy)
                    nc.sync.dma_start(
                        out=bass.AP(
                            tensor=kv_loc.tensor,
                            offset=kv_loc[:].offset + DIM * S + ta * DIM
                            + sl * SLICE,
                            ap=[[DIM, tw], [1, SLICE]]),
                        in_=vt[:tw, :])

        # ---- phase order: V first (no rms/rope chain), K next (its rope runs
        # on ACT/DVE while Q's matmuls keep the PE busy), then ONE fused
        # AllGather (the collective-bandwidth curve strongly favors a single
        # big transfer); Q's rope overlaps the gather ----
        v_proj()
        qk_proj("wkT", bk_sb, gk_sb, bkgk_sb, khat, r_k)
        rope(khat, krot, r_k, "k")
        for m in range(NH):
            nc.sync.dma_start(
                out=bass.AP(tensor=kv_loc.tensor,
                            offset=kv_loc[:].offset + m * 128 * S,
                            ap=[[S, 128], [1, S]]),
                in_=krot[:, m, :])
        qk_proj("wqT", bq_sb, gq_sb, bqgq_sb, qhat, r_q)
        rope(qhat, qrot, r_q, "q")
        nc.gpsimd.collective_compute(
            "AllGather", mybir.AluOpType.bypass, ins=[kv_loc[:]],
            outs=[kv_all[:]], replica_groups=rg)

        # ---------------- attention ----------------
        # Keys are processed in GLOBAL frame order (contiguous across source
        # cores) so every chunk is a full 128 keys (13 chunks / frame instead
        # of 16).  Per head pair: score tiles [128, 2(hi), W<=390] live in a
        # 2-bank PSUM tile (hi at col 0 / 512) so exp is ONE ACT instruction
        # covering both heads.  q blocks are frame-aligned; o and z accumulate
        # per query-frame (o: [128, 2, T] bank per qf; z: [1, 2, T] rows of a
        # shared bank), evicted as soon as kf == qf completes.
        actx = ctx.enter_context(ExitStack())
        att_k = actx.enter_context(tc.tile_pool(name="att_k", bufs=2))
        att_v = actx.enter_context(tc.tile_pool(name="att_v", bufs=2))
        att_s = actx.enter_context(tc.tile_pool(name="att_s", bufs=2, space="PSUM"))
        att_o = actx.enter_context(tc.tile_pool(name="att_o", bufs=1, space="PSUM"))
        att_z = actx.enter_context(tc.tile_pool(name="att_z", bufs=1, space="PSUM"))
        att_p = actx.enter_context(tc.tile_pool(name="att_p", bufs=6))
        att_m = actx.enter_context(tc.tile_pool(name="att_m", bufs=2))

        KCH_G = _chunks(GF, 128)          # 13 chunks of global frame keys
        NJ = len(KCH_G)
        # v-load pieces: (core, t0, t1, chunk j, partition p0) covering one frame
        vpieces = []
        for c in range(NC):
            g = c * T
            while g < (c + 1) * T:
                j = g // 128
                ge = min((c + 1) * T, (j + 1) * 128)
                vpieces.append((c, g - c * T, ge - c * T, j, g - j * 128))
                g = ge
        # q blocks per key frame: [q0, S) in pieces of <=2*T, frame-aligned
        QBLK = {}
        for kf in range(F):
            q0 = T * first_qf[kf]
            blocks = []
            while q0 < S:
                q1 = min(q0 + 2 * T, S)
                blocks.append((q0, q1))
                q0 = q1
            QBLK[kf] = blocks

        NPG = NH // 2
        for pg in range(NPG):
            o_ps = {}
            for qf in range(F):
                o_ps[qf] = att_o.tile([128, 512], dt.float32, tag=f"o{qf}",
                                      name=f"o{qf}")
            z_ps = att_z.tile([128, 512], dt.float32, tag="z", name="z")

            for kf in range(F):
                kr_t = att_k.tile([128, 2, GF], dt.bfloat16, tag="kr")
                for hi in range(2):
                    nc.sync.dma_start(
                        out=kr_t[:, hi, :],
                        in_=bass.AP(
                            tensor=kv_all.tensor,
                            offset=kv_all[:].offset
                            + ((pg * 2 + hi) * 128) * S + kf * T,
                            ap=[[S, 128], [KVSZ, NC], [1, T]]))
                v_t = att_v.tile([128, NJ, 2, 128], dt.bfloat16, tag="vt")
                for (c, t0, t1, j, p0) in vpieces:
                    nc.sync.dma_start(
                        out=v_t[p0:p0 + (t1 - t0), j, :, :],
                        in_=bass.AP(
                            tensor=kv_all.tensor,
                            offset=kv_all[:].offset + c * KVSZ + DIM * S
                            + (kf * T + t0) * DIM + pg * 256,
                            ap=[[DIM, t1 - t0], [1, 256]]))
                for j, (ga, gb) in enumerate(KCH_G):
                    kw = gb - ga
                    for (qa, qb) in QBLK[kf]:
                        W = qb - qa
                        s_t = att_s.tile([128, 1024], dt.float32, tag="s")
                        for hi in range(2):
                            nc.tensor.matmul(
                                s_t[:kw, hi * 512:hi * 512 + W],
                                kr_t[:, hi, ga:gb],
                                qrot[:, 2 * pg + hi, qa:qb],
                                start=True, stop=True)
                        p_t = att_p.tile([128, 2, 2 * T], dt.bfloat16, tag="p")
                        sv = s_t[:].rearrange("p (two x) -> p two x", two=2)
                        nc.scalar.activation(p_t[:kw, :, :W], sv[:kw, :, :W],
                                             AF.Exp, scale=inv_sqrt_d)
                        for qf in range(qa // T, (qb - 1) // T + 1):
                            oa = qf * T - qa
                            first = (kf == 0 and j == 0)
                            last = (kf == qf and j == NJ - 1)
                            nc.tensor.matmul(
                                z_ps[32 * qf:32 * qf + 1, 0:2 * T],
                                ones_key[:kw, :],
                                p_t[:kw, :, oa:oa + T],
                                start=first, stop=last)
                            for hi in range(2):
                                # start=True zeroes the whole bank row, so only
                                # hi0 opens the group; hi1 accumulates onto the
                                # zeroed right half (program order guarantees
                                # hi0's start executes first)
                                nc.tensor.matmul(
                                    o_ps[qf][:, hi * T:(hi + 1) * T],
                                    v_t[:kw, j, hi, :],
                                    p_t[:kw, hi, oa:oa + T],
                                    start=(first and hi == 0), stop=last,
                                    skip_group_check=True)
                # query frame kf is complete: normalize + evict
                qf = kf
                z_sb = att_m.tile([1, 2 * T], dt.float32, tag="zsb", name="zsb")
                z_dram = dram.tile([2, S], dt.float32, tag="zdram", bufs=2,
                                   name="zdram")
                nc.scalar.activation(z_sb[0:1, :],
                                     z_ps[32 * qf:32 * qf + 1, 0:2 * T],
                                     AF.Copy)
                nc.vector.reciprocal(z_sb[0:1, :], z_sb[0:1, :])
                nc.sync.dma_start(
                    out=bass.AP(tensor=z_dram.tensor,
                                offset=z_dram[:].offset + qf * T,
                                ap=[[S, 2], [1, T]]),
                    in_=z_sb[0:1, :])
                izb = att_m.tile([128, 2, T], dt.float32, tag="izb", name="izb")
                nc.sync.dma_start(
                    out=izb[:],
                    in_=bass.AP(tensor=z_dram.tensor,
                                offset=z_dram[:].offset + qf * T,
                                ap=[[0, 128], [S, 2], [1, T]]))
                for hi in range(2):
                    nc.vector.tensor_mul(
                        oT_sb[:, 2 * pg + hi, qf * T:(qf + 1) * T],
                        o_ps[qf][:, hi * T:(hi + 1) * T],
                        izb[:, hi, :])

        actx.close()  # release attention PSUM banks before the O-projection

        if debug:
            dbg_oT = nc.dram_tensor("dbg_oT", [128, NH * S], dt.bfloat16,
                                    kind="ExternalOutput")
            dbg_kr = nc.dram_tensor("dbg_kr", [128, NH * S], dt.bfloat16,
                                    kind="ExternalOutput")
            dbg_qr = nc.dram_tensor("dbg_qr", [128, NH * S], dt.bfloat16,
                                    kind="ExternalOutput")
            nc.sync.dma_start(out=dbg_oT[:], in_=oT_sb[:].rearrange("p m s -> p (m s)"))
            nc.sync.dma_start(out=dbg_kr[:], in_=krot[:].rearrange("p m s -> p (m s)"))
            nc.sync.dma_start(out=dbg_qr[:], in_=qrot[:].rearrange("p m s -> p (m s)"))

        # ---------------- O projection ----------------
        wpool = ctx.enter_context(tc.tile_pool(name="w_o", bufs=3))
        pspool = ctx.enter_context(
            tc.tile_pool(name="ps_o", bufs=len(TOKCH) + 1, space="PSUM"))
        evpool = ctx.enter_context(tc.tile_pool(name="ev_o", bufs=3))
        for sl in range(NSL):
            ps = {}
            for ti in range(len(TOKCH)):
                ps[ti] = pspool.tile([128, SLICE], dt.float32, tag="ops", name=f"ops{ti}")
            for m in range(NH):
                wt = wpool.tile([128, SLICE], dt.bfloat16, tag="wo")
                nc.sync.dma_start(
                    out=wt[:],
                    in_=w_d["woT"][m * 128:(m + 1) * 128,
                                   sl * SLICE:(sl + 1) * SLICE])
                for ti, (ta, tb) in enumerate(TOKCH):
                    nc.tensor.matmul(ps[ti][:tb - ta, :], oT_sb[:, m, ta:tb],
                                     wt[:], start=(m == 0), stop=(m == NH - 1))
            for ti, (ta, tb) in enumerate(TOKCH):
                tw = tb - ta
                ot = evpool.tile([128, SLICE], dt.float32, tag="oev")
                nc.vector.tensor_add(ot[:tw, :], ps[ti][:tw, :],
                                     bo_bc[:tw, sl * SLICE:(sl + 1) * SLICE])
                nc.sync.dma_start(
                    out=out_d[ta:tb, sl * SLICE:(sl + 1) * SLICE],
                    in_=ot[:tw, :])

    if cap_waits:
        _cap_sync_waits(nc, mybir)
    _BUILD_CACHE[key] = nc
    return nc


def _cap_sync_waits(nc, mybir, cap=1):
    """Walrus engine-instruction structs only have a limited number of sync
    wait slots.  Hoist excess waits onto InstNoOp carriers placed immediately
    before the instruction on the same engine stream."""
    exempt = (mybir.InstNoOp, mybir.InstEventSemaphore,
              mybir.InstAllEngineBarrier)
    for f in nc.m.functions:
        for bb in f.blocks:
            out = []
            changed = False
            for inst in bb.instructions:
                si = inst.sync_info
                if (si is None or len(si.on_wait) <= cap
                        or isinstance(inst, exempt)):
                    out.append(inst)
                    continue
                waits = list(si.on_wait)
                keep, excess = waits[:cap], waits[cap:]
                while excess:
                    batch, excess = excess[:cap], excess[cap:]
                    out.append(mybir.InstNoOp(
                        name=f"{inst.name}-w{len(out)}",
                        engine=inst.engine,
                        bass_nofuse=True,
                        sync_info=mybir.SyncInfo(on_wait=batch, on_update=[]),
                    ))
                inst.sync_info = mybir.SyncInfo(on_wait=keep,
                                                on_update=list(si.on_update))
                out.append(inst)
                changed = True
            if changed:
                bb.instructions = out


# ---------------------------------------------------------------------------
# host side
# ---------------------------------------------------------------------------
def _perm(NH):
    p = np.empty(NH * D, np.int64)
    for hh in range(NH):
        base = hh * D
        for j in range(D // 2):
            p[base + j] = base + 2 * j
            p[base + D // 2 + j] = base + 2 * j + 1
    return p


def _host_inputs(x, freqs, Wq, bq, Wk, bk, Wv, bv, Wo, bo, gq, gk,
                 f, h, w, num_heads, local_attn_size, sink_size, start_frame):
    NH = num_heads
    DIM = NH * D
    FRAME = h * w
    assert FRAME % NC == 0
    T = FRAME // NC
    S = f * T
    perm = _perm(NH)

    def bf(a):
        return np.ascontiguousarray(a, dtype=np.float32).astype(BF16)

    wqT = bf(Wq[perm].T)
    wkT = bf(Wk[perm].T)
    wvT = bf(Wv.T)
    woT = bf(Wo.T)
    def chunkmajor(a):
        return np.asarray(a, np.float32)[perm].reshape(NH, D).T
    bias_pack = np.ascontiguousarray(np.concatenate(
        [chunkmajor(bq), chunkmajor(gq), chunkmajor(bq) * chunkmajor(gq),
         chunkmajor(bk), chunkmajor(gk), chunkmajor(bk) * chunkmajor(gk)],
        axis=1), np.float32)
    bv_r = bf(bv.reshape(1, DIM))
    bo_r = np.ascontiguousarray(bo.reshape(1, DIM), np.float32)
    # half-swap permutation: sw[p] = hat[(p + 64) % 128]  (as lhsT for matmul)
    perm128 = np.zeros((128, 128), np.float32)
    for m_ in range(128):
        perm128[(m_ + 64) % 128, m_] = 1.0
    perm128 = perm128.astype(BF16)

    c = D // 2
    c1 = c // 3
    c0 = c - 2 * c1
    freqs = np.asarray(freqs, np.float32)

    in_maps = []
    tok_idx = []
    for core in range(NC):
        idx = np.concatenate(
            [fr * FRAME + T * core + np.arange(T) for fr in range(f)])
        tok_idx.append(idx)
        xT = bf(np.asarray(x[0], np.float32)[idx].T)
        fr = idx // FRAME
        rem = idx % FRAME
        hh_i = rem // w
        ww_i = rem % w
        ang = np.empty((c, S), np.float32)
        ang[:c0, :] = freqs[start_frame + fr][:, :c0].T
        ang[c0:c0 + c1, :] = freqs[hh_i][:, c0:c0 + c1].T
        ang[c0 + c1:, :] = freqs[ww_i][:, c0 + c1:c].T
        def wrap(a):
            a = np.asarray(a, np.float64)
            return (a - 2 * np.pi * np.round(a / (2 * np.pi))).astype(np.float32)
        # top half encodes -sin via the (ang + pi) phase shift
        angS = np.ascontiguousarray(
            np.concatenate([wrap(ang + np.pi), wrap(ang)], 0), np.float32)
        angC = np.ascontiguousarray(
            np.concatenate([wrap(ang + np.pi / 2), wrap(ang + np.pi / 2)], 0),
            np.float32)
        in_maps.append({
            "xT": xT, "wqT": wqT, "wkT": wkT, "wvT": wvT, "woT": woT,
            "bias_pack": bias_pack, "perm128": perm128,
            "bv_r": bv_r, "bo_r": bo_r, "angS": angS, "angC": angC,
        })
    return in_maps, tok_idx, T, S


def _allowed(f, local_attn_size, sink_size):
    return [
        [kf for kf in range(f)
         if kf <= qf and (qf - kf < local_attn_size or kf < sink_size)]
        for qf in range(f)
    ]


def kernel(x, freqs, Wq, bq, Wk, bk, Wv, bv, Wo, bo, gq, gk,
           f, h, w, num_heads, local_attn_size, sink_size, start_frame,
           _trace=False):
    from concourse.bass_utils import run_bass_kernel_spmd

    f = int(f); h = int(h); w = int(w)
    num_heads = int(num_heads)
    local_attn_size = int(local_attn_size)
    sink_size = int(sink_size)
    start_frame = int(start_frame)

    x = np.asarray(x)
    B, L, DIM = x.shape
    assert B == 1 and DIM == num_heads * D

    allowed = _allowed(f, local_attn_size, sink_size)
    in_maps, tok_idx, T, S = _host_inputs(
        x, freqs, Wq, bq, Wk, bk, Wv, bv, Wo, bo, gq, gk,
        f, h, w, num_heads, local_attn_size, sink_size, start_frame)
    nc = build_program(num_heads, f, T, allowed)
    res = run_bass_kernel_spmd(nc, in_maps, core_ids=list(range(NC)),
                               trace=_trace)
    out = np.empty((1, L, DIM), np.float32)
    for core in range(NC):
        out[0, tok_idx[core]] = res.results[core]["out"]
    if _trace:
        kernel._last_results = res
    return out



# revision 46
# speedup vs baseline: 3.6699x; 1.0620x over previous
"""Trainium2 Bass kernel for CausalWanSelfAttention (frame-causal windowed
attention with QK-RMSNorm + RoPE), sharded over 8 NeuronCores.

Sharding: each core owns T = (h*w)/8 tokens of every frame (frame-balanced
interleave).  Each core computes Q/K/V projections + RMSNorm + RoPE for its
own tokens, K/V are exchanged with two AllGathers, attention + O-projection
are computed locally for the core's query tokens.

Device layouts:
  - q/k feature-major [ch, tok] (channels on partitions), with each head's
    128 channels permuted to [re(0..63) | im(0..63)] so RoPE works on
    contiguous partition blocks (permutation is folded into Wq/Wk on host).
  - v token-major [tok, ch] (natural channel order).
  - scores computed as s^T [keys, q]  ->  softmax denominator via
    ones-matmul (partition reduction on the tensor engine), exp on ACT.
  - attention out o^T [ch, tok]; per-head 1/Z applied by DVE during PSUM
    eviction with a partition-broadcast tile.
  - RMSNorm scale r (per token) is folded into the RoPE cos/sin tables
    (scalar multiplication commutes with rotation); per-channel gain g and
    bias b are folded into the ACT eviction (per-partition scale/bias).
"""

import math
import sys
from contextlib import ExitStack

import numpy as np

if "/opt/trn_rl_repo" not in sys.path:
    sys.path.insert(0, "/opt/trn_rl_repo")

import ml_dtypes

BF16 = ml_dtypes.bfloat16
NC = 8  # cores
D = 128  # head dim
EPS = 1e-6


# ---------------------------------------------------------------------------
# helpers
# ---------------------------------------------------------------------------
def _pieces(lo, hi, T):
    """Split the global (within-frame) token range [lo, hi) into per-core
    pieces.  Returns [(core, a, b)] with a/b local to the core's frame-chunk."""
    out = []
    c = lo // T
    while lo < hi:
        b = min(hi, (c + 1) * T)
        out.append((c, lo - c * T, b - c * T))
        lo = b
        c += 1
    return out


def _segs(q0, S, bank=512):
    """Split [q0, S) at multiples of `bank` -> list of absolute (qa, qb)."""
    pts = [q0]
    nxt = (q0 // bank + 1) * bank
    while nxt < S:
        pts.append(nxt)
        nxt += bank
    pts.append(S)
    return [(pts[i], pts[i + 1]) for i in range(len(pts) - 1)]


def _chunks(frame_len, width=128):
    return [(g * width, min(frame_len, (g + 1) * width))
            for g in range((frame_len + width - 1) // width)]


# ---------------------------------------------------------------------------
# device program
# ---------------------------------------------------------------------------
_BUILD_CACHE = {}


def build_program(NH, F, T, allowed_kf, cap_waits=True, debug=False):
    """Build the SPMD Bass program (identical on all 8 cores).

    NH: number of heads; F: frames; T: tokens per (core, frame);
    allowed_kf[qf] = list of key frames query-frame qf may attend to
    (must make, for each kf, the attending q-set a contiguous suffix of
    frames -- true for causal masks).
    """
    key = (NH, F, T, tuple(tuple(a) for a in allowed_kf), cap_waits, debug)
    if key in _BUILD_CACHE:
        return _BUILD_CACHE[key]

    import concourse.bass as bass
    import concourse.mybir as mybir
    import concourse.tile as tile
    from concourse.mybir import ActivationFunctionType as AF

    dt = mybir.dt
    DIM = NH * D
    S = F * T              # tokens per core
    FRAME = NC * T         # tokens per frame
    NHALF = 2
    H0 = (S + 1) // 2      # token halves for the q/k projections
    SLICE = min(512, DIM)  # out-channel slice for v/o projections
    NSL = DIM // SLICE
    TOKCH = _chunks(S, 128)  # token chunks for v/o projections

    # for each key frame kf: the first query frame that attends to it, and
    # check the q-set is a suffix
    first_qf = {}
    for kf in range(F):
        qs = [qf for qf in range(F) if kf in allowed_kf[qf]]
        assert qs, f"key frame {kf} unused"
        assert qs == list(range(qs[0], F)), "non-suffix q-set unsupported"
        first_qf[kf] = qs[0]

    nc = bass.Bass()

    # ---------------- I/O ----------------
    xT_d = nc.dram_tensor("xT", [DIM, S], dt.bfloat16, kind="ExternalInput")
    w_d = {}
    for nm in ("wqT", "wkT", "wvT", "woT"):
        w_d[nm] = nc.dram_tensor(nm, [DIM, DIM], dt.bfloat16, kind="ExternalInput")
    # packed per-channel affine constants: bq|gq|bq*gq|bk|gk|bk*gk
    bias_d = nc.dram_tensor("bias_pack", [128, 6 * NH], dt.float32,
                            kind="ExternalInput")
    perm_d = nc.dram_tensor("perm128", [128, 128], dt.bfloat16,
                            kind="ExternalInput")
    bv_d = nc.dram_tensor("bv_r", [1, DIM], dt.bfloat16, kind="ExternalInput")
    bo_d = nc.dram_tensor("bo_r", [1, DIM], dt.float32, kind="ExternalInput")
    angS_d = nc.dram_tensor("angS", [128, S], dt.float32, kind="ExternalInput")
    angC_d = nc.dram_tensor("angC", [128, S], dt.float32, kind="ExternalInput")
    out_d = nc.dram_tensor("out", [S, DIM], dt.float32, kind="ExternalOutput")

    rg = [list(range(NC))]
    inv_sqrt_d = 1.0 / math.sqrt(D)

    GF = NC * T            # global keys per frame (1560)
    KVSZ = 2 * DIM * S     # flat kv block per core (k feature-major + v token-major)

    with tile.TileContext(nc) as tc, ExitStack() as ctx:
        dram = ctx.enter_context(tc.tile_pool(name="dram", bufs=1, space="DRAM"))
        kv_loc = dram.tile([1, KVSZ], dt.bfloat16)
        kv_all = dram.tile([NC, KVSZ], dt.bfloat16, addr_space="Shared")

        const = ctx.enter_context(tc.tile_pool(name="const", bufs=1))
        resid = ctx.enter_context(tc.tile_pool(name="resid", bufs=1))

        ones_key = const.tile([128, 1], dt.bfloat16)
        nc.vector.memset(ones_key, 1.0)
        ones_row = const.tile([1, 128], dt.bfloat16)
        nc.vector.memset(ones_row, 1.0)
        perm_sb = const.tile([128, 128], dt.bfloat16)
        nc.sync.dma_start(out=perm_sb[:], in_=perm_d[:])
        negpi = const.tile([128, 1], dt.float32)
        nc.vector.memset(negpi, -math.pi)
        neg2 = const.tile([128, 1], dt.float32)
        nc.vector.memset(neg2, -2.0)
        eps_t = const.tile([128, 1], dt.float32)
        nc.vector.memset(eps_t, EPS)

        # constant / bias tiles (one DMA for the packed affine constants)
        bias_sb = const.tile([128, 6 * NH], dt.float32)
        nc.sync.dma_start(out=bias_sb[:], in_=bias_d[:])
        bq_sb = bias_sb[:, 0 * NH:1 * NH]
        gq_sb = bias_sb[:, 1 * NH:2 * NH]
        bqgq_sb = bias_sb[:, 2 * NH:3 * NH]
        bk_sb = bias_sb[:, 3 * NH:4 * NH]
        gk_sb = bias_sb[:, 4 * NH:5 * NH]
        bkgk_sb = bias_sb[:, 5 * NH:6 * NH]
        bv_sb = const.tile([1, DIM], dt.bfloat16)
        nc.sync.dma_start(out=bv_sb[:], in_=bv_d[:])
        bo_bc = const.tile([128, DIM], dt.float32)
        nc.sync.dma_start(
            out=bo_bc[:],
            in_=bass.AP(tensor=bo_d[:].tensor, offset=bo_d[:].offset,
                        ap=[[0, 128]] + bo_d[:].ap[1:]),
        )

        # x (feature-major), resident
        xT_sb = resid.tile([128, NH, S], dt.bfloat16)
        nc.sync.dma_start(out=xT_sb[:, :, :S],
                          in_=xT_d[:].rearrange("(m p) s -> p m s", p=128))

        # raw RoPE sin/cos (shared q/k)
        angS_sb = resid.tile([128, S], dt.float32)
        angC_sb = resid.tile([128, S], dt.float32)
        nc.sync.dma_start(out=angS_sb[:], in_=angS_d[:])
        nc.sync.dma_start(out=angC_sb[:], in_=angC_d[:])
        # angles arrive host-canonicalized to [-pi, pi] (ACT Sin table range)
        sin_raw = resid.tile([128, S], dt.float32)
        cos_raw = resid.tile([128, S], dt.float32)
        nc.scalar.activation(sin_raw[:], angS_sb[:], AF.Sin)
        nc.scalar.activation(cos_raw[:], angC_sb[:], AF.Sin)

        qhat = resid.tile([128, NH, S], dt.bfloat16)
        khat = resid.tile([128, NH, S], dt.bfloat16)
        qrot = resid.tile([128, NH, S], dt.bfloat16)
        krot = resid.tile([128, NH, S], dt.bfloat16)
        r_q = resid.tile([1, S], dt.bfloat16)
        r_k = resid.tile([1, S], dt.bfloat16)
        oT_sb = resid.tile([128, NH, S], dt.bfloat16)

        halves = [(0, H0), (H0, S)] if S > H0 else [(0, S)]

        # ---------------- Q/K projections + RMS stats ----------------
        def qk_proj(wname, bias_sb, gain_sb, bg_sb, hat, r_sb):
          with ExitStack() as pctx:
            wpool = pctx.enter_context(tc.tile_pool(name=f"w_{wname}", bufs=3))
            pspool = pctx.enter_context(
                tc.tile_pool(name=f"ps_{wname}", bufs=4, space="PSUM"))
            sspool = pctx.enter_context(
                tc.tile_pool(name=f"ss_{wname}", bufs=2, space="PSUM"))
            evpool = pctx.enter_context(tc.tile_pool(name=f"ev_{wname}", bufs=3))
            wsrc = w_d[wname][:].rearrange("(kc p) n -> p kc n", p=128)
            ss_ps = {}
            for hi, (ha, hb) in enumerate(halves):
                ss_ps[hi] = sspool.tile([1, hb - ha], dt.float32, tag="ss", name=f"ss{hi}")
            for m in range(NH):
                # per-head weight slice: small DMAs prefetch via pool rotation
                w_m = wpool.tile([128, NH, 128], dt.bfloat16, tag="w")
                nc.sync.dma_start(out=w_m[:],
                                  in_=wsrc[:, :, m * 128:(m + 1) * 128])
                ps = {}
                for hi, (ha, hb) in enumerate(halves):
                    ps[hi] = pspool.tile([128, hb - ha], dt.float32, tag="ps", name=f"ps{hi}")
                for kc in range(NH):
                    for hi, (ha, hb) in enumerate(halves):
                        nc.tensor.matmul(ps[hi][:, :hb - ha],
                                         w_m[:, kc, :],
                                         xT_sb[:, kc, ha:hb],
                                         start=(kc == 0), stop=(kc == NH - 1))
                for hi, (ha, hb) in enumerate(halves):
                    hw_ = hb - ha
                    sq = evpool.tile([128, H0], dt.bfloat16, tag="sq")
                    # (q + b)^2
                    nc.scalar.activation(sq[:, :hw_], ps[hi][:, :hw_], AF.Square,
                                         bias=bias_sb[:, m:m + 1])
                    # qhat = (q + b) * g = q*g + b*g
                    nc.scalar.activation(hat[:, m, ha:hb], ps[hi][:, :hw_],
                                         AF.Identity, bias=bg_sb[:, m:m + 1],
                                         scale=gain_sb[:, m:m + 1])
                    nc.tensor.matmul(ss_ps[hi][0:1, :hw_], ones_key[:],
                                     sq[:, :hw_],
                                     start=(m == 0), stop=(m == NH - 1))
            for hi, (ha, hb) in enumerate(halves):
                hw_ = hb - ha
                rt = evpool.tile([1, H0], dt.float32, tag="rt")
                # sqrt(mean(q^2) + eps)
                nc.scalar.activation(rt[0:1, :hw_], ss_ps[hi][0:1, :hw_], AF.Sqrt,
                                     bias=eps_t[0:1, :], scale=1.0 / DIM)
                with nc.allow_low_precision(reason="1/rms folded into bf16 rope tables"):
                    nc.vector.reciprocal(r_sb[0:1, ha:hb], rt[0:1, :hw_])

        # ---------------- RoPE ----------------
        # The half-swap and the 1/rms broadcast both run on the (otherwise
        # idle) tensor engine: sw = P @ hat with a host-provided permutation,
        # rb = ones^T @ r.  Matmul outputs split at the 512-col psum bank edge.
        PSEG = [(a, min(a + 512, S)) for a in range(0, S, 512)]

        def rope(hat, rot, r_sb, tag):
          with ExitStack() as pctx:
            rp = pctx.enter_context(tc.tile_pool(name=f"rope_{tag}", bufs=3))
            rps = pctx.enter_context(
                tc.tile_pool(name=f"rps_{tag}", bufs=1, space="PSUM"))
            rb_ps = rps.tile([128, 1024], dt.float32, tag="rb", name=f"rb_{tag}")
            for (a, b) in PSEG:
                nc.tensor.matmul(rb_ps[:, a:b], ones_row[:], r_sb[0:1, a:b],
                                 start=True, stop=True)
            ct = resid.tile([128, S], dt.bfloat16, name=f"cos_{tag}")
            st = resid.tile([128, S], dt.bfloat16, name=f"sin_{tag}")
            nc.vector.tensor_mul(ct[:], cos_raw[:], rb_ps[:, :S])
            nc.vector.tensor_mul(st[:], sin_raw[:], rb_ps[:, :S])
            for m in range(NH):
                sw_ps = rps.tile([128, 1024], dt.float32, tag="sw",
                                 name=f"sw_{tag}{m}", bufs=2)
                for (a, b) in PSEG:
                    nc.tensor.matmul(sw_ps[:, a:b], perm_sb[:],
                                     hat[:, m, a:b], start=True, stop=True)
                t1 = rp.tile([128, S], dt.bfloat16, tag="t1")
                t2 = rp.tile([128, S], dt.bfloat16, tag="t2")
                nc.vector.tensor_mul(t1[:], hat[:, m, :], ct[:])
                nc.vector.tensor_mul(t2[:], sw_ps[:, :S], st[:])
                nc.vector.tensor_add(rot[:, m, :], t1[:], t2[:])

        # ---------------- V projection (token-major) ----------------
        def v_proj():
          with ExitStack() as pctx:
            wpool = pctx.enter_context(tc.tile_pool(name="w_v", bufs=2))
            pspool = pctx.enter_context(
                tc.tile_pool(name="ps_v", bufs=len(TOKCH) + 1, space="PSUM"))
            evpool = pctx.enter_context(tc.tile_pool(name="ev_v", bufs=3))
            wvsrc = w_d["wvT"][:].rearrange("(kc p) n -> p kc n", p=128)
            for sl in range(NSL):
                w_sb = wpool.tile([128, NH, SLICE], dt.bfloat16, tag="wv")
                nc.sync.dma_start(
                    out=w_sb[:],
                    in_=wvsrc[:, :, sl * SLICE:(sl + 1) * SLICE])
                ps = {}
                for ti in range(len(TOKCH)):
                    ps[ti] = pspool.tile([128, SLICE], dt.float32, tag="vps", name=f"vps{ti}")
                for kc in range(NH):
                    for ti, (ta, tb) in enumerate(TOKCH):
                        nc.tensor.matmul(ps[ti][:tb - ta, :],
                                         xT_sb[:, kc, ta:tb],
                                         w_sb[:, kc, :],
                                         start=(kc == 0), stop=False)
                for ti, (ta, tb) in enumerate(TOKCH):
                    tw = tb - ta
                    nc.tensor.matmul(ps[ti][:tw, :], ones_row[0:1, :tw],
                                     bv_sb[0:1, sl * SLICE:(sl + 1) * SLICE],
                                     start=False, stop=True)
                    vt = evpool.tile([128, SLICE], dt.bfloat16, tag="vev")
                    nc.scalar.activation(vt[:tw, :], ps[ti][:tw, :], AF.Copy)
                    nc.sync.dma_start(
                        out=bass.AP(
                            tensor=kv_loc.tensor,
                            offset=kv_loc[:].offset + DIM * S + ta * DIM
                            + sl * SLICE,
                            ap=[[DIM, tw], [1, SLICE]]),
                        in_=vt[:tw, :])

        # ---- phase order: V first (no rms/rope chain), K next (its rope runs
        # on ACT/DVE while Q's matmuls keep the PE busy), then ONE fused
        # AllGather (the collective-bandwidth curve strongly favors a single
        # big transfer); Q's rope overlaps the gather ----
        v_proj()
        qk_proj("wkT", bk_sb, gk_sb, bkgk_sb, khat, r_k)
        rope(khat, krot, r_k, "k")
        for m in range(NH):
            nc.sync.dma_start(
                out=bass.AP(tensor=kv_loc.tensor,
                            offset=kv_loc[:].offset + m * 128 * S,
                            ap=[[S, 128], [1, S]]),
                in_=krot[:, m, :])
        qk_proj("wqT", bq_sb, gq_sb, bqgq_sb, qhat, r_q)
        rope(qhat, qrot, r_q, "q")
        nc.gpsimd.collective_compute(
            "AllGather", mybir.AluOpType.bypass, ins=[kv_loc[:]],
            outs=[kv_all[:]], replica_groups=rg)

        # ---------------- attention ----------------
        # Keys are processed in GLOBAL frame order (contiguous across source
        # cores) so every chunk is a full 128 keys (13 chunks / frame instead
        # of 16).  Per head pair: score tiles [128, 2(hi), W<=390] live in a
        # 2-bank PSUM tile (hi at col 0 / 512) so exp is ONE ACT instruction
        # covering both heads.  q blocks are frame-aligned; o and z accumulate
        # per query-frame (o: [128, 2, T] bank per qf; z: [1, 2, T] rows of a
        # shared bank), evicted as soon as kf == qf completes.
        actx = ctx.enter_context(ExitStack())
        att_k = actx.enter_context(tc.tile_pool(name="att_k", bufs=2))
        att_v = actx.enter_context(tc.tile_pool(name="att_v", bufs=2))
        att_s = actx.enter_context(tc.tile_pool(name="att_s", bufs=2, space="PSUM"))
        att_o = actx.enter_context(tc.tile_pool(name="att_o", bufs=1, space="PSUM"))
        att_z = actx.enter_context(tc.tile_pool(name="att_z", bufs=1, space="PSUM"))
        att_p = actx.enter_context(tc.tile_pool(name="att_p", bufs=6))
        att_m = actx.enter_context(tc.tile_pool(name="att_m", bufs=2))

        KCH_G = _chunks(GF, 128)          # 13 chunks of global frame keys
        NJ = len(KCH_G)
        # v-load pieces: (core, t0, t1, chunk j, partition p0) covering one frame
        vpieces = []
        for c in range(NC):
            g = c * T
            while g < (c + 1) * T:
                j = g // 128
                ge = min((c + 1) * T, (j + 1) * 128)
                vpieces.append((c, g - c * T, ge - c * T, j, g - j * 128))
                g = ge
        # q blocks per key frame: [q0, S) in pieces of <=2*T, frame-aligned
        QBLK = {}
        for kf in range(F):
            q0 = T * first_qf[kf]
            blocks = []
            while q0 < S:
                q1 = min(q0 + 2 * T, S)
                blocks.append((q0, q1))
                q0 = q1
            QBLK[kf] = blocks

        NPG = NH // 2
        for pg in range(NPG):
            o_ps = {}
            for qf in range(F):
                o_ps[qf] = att_o.tile([128, 512], dt.float32, tag=f"o{qf}",
                                      name=f"o{qf}")
            z_ps = att_z.tile([128, 512], dt.float32, tag="z", name="z")

            for kf in range(F):
                kr_t = att_k.tile([128, 2, GF], dt.bfloat16, tag="kr")
                for hi in range(2):
                    nc.sync.dma_start(
                        out=kr_t[:, hi, :],
                        in_=bass.AP(
                            tensor=kv_all.tensor,
                            offset=kv_all[:].offset
                            + ((pg * 2 + hi) * 128) * S + kf * T,
                            ap=[[S, 128], [KVSZ, NC], [1, T]]))
                v_t = att_v.tile([128, NJ, 2, 128], dt.bfloat16, tag="vt")
                for (c, t0, t1, j, p0) in vpieces:
                    nc.sync.dma_start(
                        out=v_t[p0:p0 + (t1 - t0), j, :, :],
                        in_=bass.AP(
                            tensor=kv_all.tensor,
                            offset=kv_all[:].offset + c * KVSZ + DIM * S
                            + (kf * T + t0) * DIM + pg * 256,
                            ap=[[DIM, t1 - t0], [1, 256]]))
                # attention p/v stay bf16: fp8 p/v costs ~4% output error
                # (peaked softmax rows don't average quantization noise away)
                for j, (ga, gb) in enumerate(KCH_G):
                    kw = gb - ga
                    for (qa, qb) in QBLK[kf]:
                        W = qb - qa
                        s_t = att_s.tile([128, 1024], dt.float32, tag="s")
                        for hi in range(2):
                            nc.tensor.matmul(
                                s_t[:kw, hi * 512:hi * 512 + W],
                                kr_t[:, hi, ga:gb],
                                qrot[:, 2 * pg + hi, qa:qb],
                                start=True, stop=True)
                        p_t = att_p.tile([128, 2, 2 * T], dt.bfloat16, tag="p")
                        sv = s_t[:].rearrange("p (two x) -> p two x", two=2)
                        nc.scalar.activation(p_t[:kw, :, :W], sv[:kw, :, :W],
                                             AF.Exp, scale=inv_sqrt_d,
                                             bias=neg2[:kw, :])
                        for qf in range(qa // T, (qb - 1) // T + 1):
                            oa = qf * T - qa
                            first = (kf == 0 and j == 0)
                            last = (kf == qf and j == NJ - 1)
                            nc.tensor.matmul(
                                z_ps[32 * qf:32 * qf + 1, 0:2 * T],
                                ones_key[:kw, :],
                                p_t[:kw, :, oa:oa + T],
                                start=first, stop=last)
                            for hi in range(2):
                                # start=True zeroes the whole bank row, so only
                                # hi0 opens the group; hi1 accumulates onto the
                                # zeroed region (program order guarantees hi0's
                                # start executes first)
                                nc.tensor.matmul(
                                    o_ps[qf][:, hi * T:(hi + 1) * T],
                                    v_t[:kw, j, hi, :],
                                    p_t[:kw, hi, oa:oa + T],
                                    start=(first and hi == 0), stop=last,
                                    skip_group_check=True)
                # query frame kf is complete: normalize + evict
                qf = kf
                z_sb = att_m.tile([1, 2 * T], dt.float32, tag="zsb", name="zsb")
                z_dram = dram.tile([2, S], dt.float32, tag="zdram", bufs=2,
                                   name="zdram")
                nc.scalar.activation(z_sb[0:1, :],
                                     z_ps[32 * qf:32 * qf + 1, 0:2 * T],
                                     AF.Copy)
                nc.vector.reciprocal(z_sb[0:1, :], z_sb[0:1, :])
                nc.sync.dma_start(
                    out=bass.AP(tensor=z_dram.tensor,
                                offset=z_dram[:].offset + qf * T,
                                ap=[[S, 2], [1, T]]),
                    in_=z_sb[0:1, :])
                izb = att_m.tile([128, 2, T], dt.float32, tag="izb", name="izb")
                nc.sync.dma_start(
                    out=izb[:],
                    in_=bass.AP(tensor=z_dram.tensor,
                                offset=z_dram[:].offset + qf * T,
                                ap=[[0, 128], [S, 2], [1, T]]))
                for hi in range(2):
                    nc.vector.tensor_mul(
                        oT_sb[:, 2 * pg + hi, qf * T:(qf + 1) * T],
                        o_ps[qf][:, hi * T:(hi + 1) * T],
                        izb[:, hi, :])

        actx.close()  # release attention PSUM banks before the O-projection

        if debug:
            dbg_oT = nc.dram_tensor("dbg_oT", [128, NH * S], dt.bfloat16,
                                    kind="ExternalOutput")
            dbg_kr = nc.dram_tensor("dbg_kr", [128, NH * S], dt.bfloat16,
                                    kind="ExternalOutput")
            dbg_qr = nc.dram_tensor("dbg_qr", [128, NH * S], dt.bfloat16,
                                    kind="ExternalOutput")
            nc.sync.dma_start(out=dbg_oT[:], in_=oT_sb[:].rearrange("p m s -> p (m s)"))
            nc.sync.dma_start(out=dbg_kr[:], in_=krot[:].rearrange("p m s -> p (m s)"))
            nc.sync.dma_start(out=dbg_qr[:], in_=qrot[:].rearrange("p m s -> p (m s)"))

        # ---------------- O projection ----------------
        wpool = ctx.enter_context(tc.tile_pool(name="w_o", bufs=3))
        pspool = ctx.enter_context(
            tc.tile_pool(name="ps_o", bufs=len(TOKCH) + 1, space="PSUM"))
        evpool = ctx.enter_context(tc.tile_pool(name="ev_o", bufs=3))
        for sl in range(NSL):
            ps = {}
            for ti in range(len(TOKCH)):
                ps[ti] = pspool.tile([128, SLICE], dt.float32, tag="ops", name=f"ops{ti}")
            for m in range(NH):
                wt = wpool.tile([128, SLICE], dt.bfloat16, tag="wo")
                nc.sync.dma_start(
                    out=wt[:],
                    in_=w_d["woT"][m * 128:(m + 1) * 128,
                                   sl * SLICE:(sl + 1) * SLICE])
                for ti, (ta, tb) in enumerate(TOKCH):
                    nc.tensor.matmul(ps[ti][:tb - ta, :], oT_sb[:, m, ta:tb],
                                     wt[:], start=(m == 0), stop=(m == NH - 1))
            for ti, (ta, tb) in enumerate(TOKCH):
                tw = tb - ta
                ot = evpool.tile([128, SLICE], dt.float32, tag="oev")
                nc.vector.tensor_add(ot[:tw, :], ps[ti][:tw, :],
                                     bo_bc[:tw, sl * SLICE:(sl + 1) * SLICE])
                nc.sync.dma_start(
                    out=out_d[ta:tb, sl * SLICE:(sl + 1) * SLICE],
                    in_=ot[:tw, :])

    if cap_waits:
        _cap_sync_waits(nc, mybir)
    _BUILD_CACHE[key] = nc
    return nc


def _cap_sync_waits(nc, mybir, cap=1):
    """Walrus engine-instruction structs only have a limited number of sync
    wait slots.  Hoist excess waits onto InstNoOp carriers placed immediately
    before the instruction on the same engine stream."""
    exempt = (mybir.InstNoOp, mybir.InstEventSemaphore,
              mybir.InstAllEngineBarrier)
    for f in nc.m.functions:
        for bb in f.blocks:
            out = []
            changed = False
            for inst in bb.instructions:
                si = inst.sync_info
                if (si is None or len(si.on_wait) <= cap
                        or isinstance(inst, exempt)):
                    out.append(inst)
                    continue
                waits = list(si.on_wait)
                keep, excess = waits[:cap], waits[cap:]
                while excess:
                    batch, excess = excess[:cap], excess[cap:]
                    out.append(mybir.InstNoOp(
                        name=f"{inst.name}-w{len(out)}",
                        engine=inst.engine,
                        bass_nofuse=True,
                        sync_info=mybir.SyncInfo(on_wait=batch, on_update=[]),
                    ))
                inst.sync_info = mybir.SyncInfo(on_wait=keep,
                                                on_update=list(si.on_update))
                out.append(inst)
                changed = True
            if changed:
                bb.instructions = out


# ---------------------------------------------------------------------------
# host side
# ---------------------------------------------------------------------------
def _perm(NH):
    p = np.empty(NH * D, np.int64)
    for hh in range(NH):
        base = hh * D
        for j in range(D // 2):
            p[base + j] = base + 2 * j
            p[base + D // 2 + j] = base + 2 * j + 1
    return p


def _host_inputs(x, freqs, Wq, bq, Wk, bk, Wv, bv, Wo, bo, gq, gk,
                 f, h, w, num_heads, local_attn_size, sink_size, start_frame):
    NH = num_heads
    DIM = NH * D
    FRAME = h * w
    assert FRAME % NC == 0
    T = FRAME // NC
    S = f * T
    perm = _perm(NH)

    def bf(a):
        return np.ascontiguousarray(a, dtype=np.float32).astype(BF16)

    wqT = bf(Wq[perm].T)
    wkT = bf(Wk[perm].T)
    wvT = bf(Wv.T)
    woT = bf(Wo.T)
    def chunkmajor(a):
        return np.asarray(a, np.float32)[perm].reshape(NH, D).T
    bias_pack = np.ascontiguousarray(np.concatenate(
        [chunkmajor(bq), chunkmajor(gq), chunkmajor(bq) * chunkmajor(gq),
         chunkmajor(bk), chunkmajor(gk), chunkmajor(bk) * chunkmajor(gk)],
        axis=1), np.float32)
    bv_r = bf(bv.reshape(1, DIM))
    bo_r = np.ascontiguousarray(bo.reshape(1, DIM), np.float32)
    # half-swap permutation: sw[p] = hat[(p + 64) % 128]  (as lhsT for matmul)
    perm128 = np.zeros((128, 128), np.float32)
    for m_ in range(128):
        perm128[(m_ + 64) % 128, m_] = 1.0
    perm128 = perm128.astype(BF16)

    c = D // 2
    c1 = c // 3
    c0 = c - 2 * c1
    freqs = np.asarray(freqs, np.float32)

    in_maps = []
    tok_idx = []
    for core in range(NC):
        idx = np.concatenate(
            [fr * FRAME + T * core + np.arange(T) for fr in range(f)])
        tok_idx.append(idx)
        xT = bf(np.asarray(x[0], np.float32)[idx].T)
        fr = idx // FRAME
        rem = idx % FRAME
        hh_i = rem // w
        ww_i = rem % w
        ang = np.empty((c, S), np.float32)
        ang[:c0, :] = freqs[start_frame + fr][:, :c0].T
        ang[c0:c0 + c1, :] = freqs[hh_i][:, c0:c0 + c1].T
        ang[c0 + c1:, :] = freqs[ww_i][:, c0 + c1:c].T
        def wrap(a):
            a = np.asarray(a, np.float64)
            return (a - 2 * np.pi * np.round(a / (2 * np.pi))).astype(np.float32)
        # top half encodes -sin via the (ang + pi) phase shift
        angS = np.ascontiguousarray(
            np.concatenate([wrap(ang + np.pi), wrap(ang)], 0), np.float32)
        angC = np.ascontiguousarray(
            np.concatenate([wrap(ang + np.pi / 2), wrap(ang + np.pi / 2)], 0),
            np.float32)
        in_maps.append({
            "xT": xT, "wqT": wqT, "wkT": wkT, "wvT": wvT, "woT": woT,
            "bias_pack": bias_pack, "perm128": perm128,
            "bv_r": bv_r, "bo_r": bo_r, "angS": angS, "angC": angC,
        })
    return in_maps, tok_idx, T, S


def _allowed(f, local_attn_size, sink_size):
    return [
        [kf for kf in range(f)
         if kf <= qf and (qf - kf < local_attn_size or kf < sink_size)]
        for qf in range(f)
    ]


def kernel(x, freqs, Wq, bq, Wk, bk, Wv, bv, Wo, bo, gq, gk,
           f, h, w, num_heads, local_attn_size, sink_size, start_frame,
           _trace=False):
    from concourse.bass_utils import run_bass_kernel_spmd

    f = int(f); h = int(h); w = int(w)
    num_heads = int(num_heads)
    local_attn_size = int(local_attn_size)
    sink_size = int(sink_size)
    start_frame = int(start_frame)

    x = np.asarray(x)
    B, L, DIM = x.shape
    assert B == 1 and DIM == num_heads * D

    allowed = _allowed(f, local_attn_size, sink_size)
    in_maps, tok_idx, T, S = _host_inputs(
        x, freqs, Wq, bq, Wk, bk, Wv, bv, Wo, bo, gq, gk,
        f, h, w, num_heads, local_attn_size, sink_size, start_frame)
    nc = build_program(num_heads, f, T, allowed)
    res = run_bass_kernel_spmd(nc, in_maps, core_ids=list(range(NC)),
                               trace=_trace)
    out = np.empty((1, L, DIM), np.float32)
    for core in range(NC):
        out[0, tok_idx[core]] = res.results[core]["out"]
    if _trace:
        kernel._last_results = res
    return out

